# revision 54
# baseline (speedup 1.0000x reference)
"""Causal self-attention (transformer block) on 8 trn2 NeuronCores.

Data-parallel over batch: core i processes batch element i (B=8).
Per-core dataflow (T=1024, C=768, H=12 heads, hd=64), fp16 matmul
operands with fp32 PSUM accumulation:

  x [T,C] --PE transpose (f32)--> xT [C,T] f16    (feature-major)
  qkT[m]  = W_attn[:,m].T @ xT + b  [1536,T]      (feature-major q,k)
  v[t]    = xT[t].T @ W_attn[:,v]   [T,768+ones]  (row-major; v-bias is
            folded into the output bias: softmax rows sum to 1, so
            y = sum a*(v+b_v) = sum a*v + b_v, and b_v^T W_proj is a
            constant row added to the projection bias)
  S^T[j,i] = sum_d kT[d,j] qT[d,i]                (scores transposed,
             only causally-live 128-col strips computed)
  E = exp(S^T*scale); diag blocks masked in-place (gpsimd affine_select)
  psY += [v_j | 1].T @ E_j   (partial width; row 64 = softmax denom)
  yT = psY[0:64] * bcast(1/psY[64])               (per-half-pair norm)
  out[t] = yT[:,t].T @ W_proj + Bout, DMA out

DMA order W_attn -> x -> W_proj is enforced by a shared staging ring
(slot reuse serializes), so the qk GEMM starts ~10us earlier than with
bandwidth-shared concurrent loads.
"""
import numpy as np

import concourse.bass as bass
import concourse.tile as tile
from concourse import bacc, mybir
from concourse.bass_utils import run_bass_kernel_spmd
from concourse.masks import make_identity

f32 = mybir.dt.float32
f16 = mybir.dt.float16
Exp = mybir.ActivationFunctionType.Exp
Ident = mybir.ActivationFunctionType.Identity

B = 8
T = 1024
C = 768
H = 12
HD = 64
SCALE = HD ** -0.5
KC = C // 128        # 6 feature chunks
MT = T // 128        # 8 token tiles
GW = 512             # Tq group width
NG = T // GW         # 2 groups
VW = H * (HD + 1)    # v tile width incl. ones column (780)


def build_nc():
    nc = bacc.Bacc(None)
    x = nc.dram_tensor("x", [T, C], f32, kind="ExternalInput")
    W_attn = nc.dram_tensor("W_attn", [C, 3 * C], f32, kind="ExternalInput")
    b_attn = nc.dram_tensor("b_attn", [3 * C], f32, kind="ExternalInput")
    W_proj = nc.dram_tensor("W_proj", [C, C], f32, kind="ExternalInput")
    b_proj = nc.dram_tensor("b_proj", [C], f32, kind="ExternalInput")
    out = nc.dram_tensor("out", [T, C], f32, kind="ExternalOutput")

    with tile.TileContext(nc) as tc:
        with (
            tc.tile_pool(name="consts", bufs=1) as consts,
            tc.tile_pool(name="stage", bufs=1) as stage,
            tc.tile_pool(name="stageW", bufs=4) as stageW,
            tc.tile_pool(name="stageX", bufs=3) as stageX,
            tc.tile_pool(name="wq", bufs=1) as wq,
            tc.tile_pool(name="wp", bufs=1) as wp,
            tc.tile_pool(name="big", bufs=1) as big,
            tc.tile_pool(name="ep", bufs=4) as ep,
            tc.tile_pool(name="recp", bufs=2) as recp,
            tc.tile_pool(name="outp", bufs=2) as outp,
            tc.tile_pool(name="psG", bufs=2, space="PSUM") as psG,
            tc.tile_pool(name="psA", bufs=2, space="PSUM") as psA,
            tc.tile_pool(name="psY", bufs=2, space="PSUM") as psY,
        ):
            # ---- constants ----
            ident32 = consts.tile([128, 128], f32, tag="ident32")
            make_identity(nc, ident32[:, :])
            ones16 = consts.tile([1, 128], f16, tag="ones16")
            nc.vector.memset(ones16[:, :], 1.0)
            ones65 = consts.tile([65, 64], f16, tag="ones65")
            nc.vector.memset(ones65[:, :], 1.0)

            # qk bias, one column per 128-feature chunk (per-partition add).
            # Loaded contiguously as [18,128] + one PE transpose — the
            # direct [128,18] load would be 2304 scattered 4-byte DMA
            # descriptors ahead of the weight DMAs.
            NB = 3 * C // 128
            bst = stage.tile([NB, 128], f32, tag="bst")
            nc.sync.dma_start(
                out=bst[:, :],
                in_=b_attn.ap().rearrange("(m p) -> m p", p=128))
            bcol = consts.tile([128, NB], f32, tag="bcol")
            psbt = psG.tile([128, NB], f32, tag="g", name="psbt")
            nc.tensor.transpose(psbt[:, :], bst[:, :], ident32[0:NB, 0:NB])
            nc.scalar.copy(bcol[:, :], psbt[:, :])
            bp32 = stage.tile([1, C], f32, tag="bp32")
            nc.sync.dma_start(out=bp32[:, :], in_=b_proj.ap().rearrange("(a d) -> a d", a=1))

            # ---- v tiles + their ones columns first: no data deps, so the
            # gpsimd queue isn't blocked behind DMA-gated weight converts
            v_sb = [big.tile([128, VW], f16, tag=f"v_{t}", name=f"v_{t}")
                    for t in range(MT)]
            for t in range(MT):
                vht_ = v_sb[t][:, :].rearrange("p (h s) -> p h s", s=HD + 1)
                nc.gpsimd.memset(vht_[:, :, HD:HD + 1], 1.0)

            # ---- staged loads (W first in issue order, then x, then W_proj)
            W16 = []
            # xT split in half-T tiles so group-0 consumers don't wait on
            # token tiles 4..7
            xTh = [big.tile([128, KC, T // 2], f16, tag=f"xT{i}", name=f"xT{i}")
                   for i in range(2)]

            def xTap(k, lo, hi):
                if hi <= T // 2:
                    return xTh[0][:, k, lo:hi]
                return xTh[1][:, k, lo - T // 2:hi - T // 2]

            for k in range(KC):
                w32 = stageW.tile([128, 3 * C], f32, tag="w32", name=f"w32_{k}")
                nc.sync.dma_start(out=w32[:, :], in_=W_attn[k * 128:(k + 1) * 128, :])
                wt = wq.tile([128, 3 * C], f16, tag=f"W16_{k}", name=f"W16_{k}")
                nc.scalar.copy(wt[:, 0:3 * C // 2], w32[:, 0:3 * C // 2])
                nc.vector.tensor_copy(wt[:, 3 * C // 2:], w32[:, 3 * C // 2:])
                W16.append(wt)
            def emit_transpose(t, x32):
                for half in range(2):
                    pst = psG.tile([128, 3, 128], f32, tag="g", name="pst")
                    for c in range(3):
                        cc = half * 3 + c
                        nc.tensor.transpose(
                            pst[:, c, :], x32[:, cc * 128:(cc + 1) * 128],
                            ident32[:, :])
                    tt = (t % 4) * 128
                    dst = xTh[t // 4][:, half * 3:half * 3 + 3, tt:tt + 128]
                    if half == 0 and t < 4:
                        nc.scalar.copy(dst, pst[:, :, :])
                    else:
                        nc.vector.tensor_copy(dst, pst[:, :, :])

            x32_late = {}
            for t in range(MT):
                # tiles 4..7 go through the W ring: the slot-reuse dep holds
                # their DMAs back so W_attn + x0..3 own the early bandwidth.
                # Their transposes are deferred into phase B so the in-order
                # PE queue isn't blocked on the late DMAs.
                pool = stageX if t < 4 else stageW
                x32 = pool.tile([128, C], f32,
                                tag="x32" if t < 4 else "w32",
                                name=f"x32_{t}")
                nc.sync.dma_start(out=x32[:, :], in_=x[t * 128:(t + 1) * 128, :])
                emit_transpose(t, x32)
            # W_proj staged through the same ring as W_attn: the slot-reuse
            # dependency keeps these DMAs off the wire until the W_attn
            # conversions are done (W_proj isn't needed until phase C).
            wp32s = []
            for k in range(KC):
                wp32 = stageW.tile([128, C], f32, tag="w32", name=f"wp32_{k}")
                nc.sync.dma_start(out=wp32[:, :], in_=W_proj[k * 128:(k + 1) * 128, :])
                wp32s.append(wp32)

            # ---- qk^T GEMM: qkT[m] [128, T] f16, m 0..5 = q, 6..11 = k
            qkT = [big.tile([128, T], f16, tag=f"qkT_{m}", name=f"qkT_{m}")
                   for m in range(12)]

            def emit_qk(m, n, on_vector=False):
                ps = psG.tile([128, 512], f32, tag="g", name="qk_ps")
                for k in range(KC):
                    nc.tensor.matmul(
                        ps[:, :], W16[k][:, m * 128:(m + 1) * 128],
                        xTap(k, n * 512, (n + 1) * 512),
                        start=(k == 0), stop=(k == KC - 1))
                dst = qkT[m][:, n * 512:(n + 1) * 512]
                if on_vector:
                    nc.vector.tensor_scalar_add(dst, ps[:, :], bcol[:, m:m + 1])
                else:
                    nc.scalar.activation(dst, ps[:, :], Ident, bias=bcol[:, m:m + 1])

            # ---- v rows: v_sb[t] [128, 780] f16 (64 cols + ones col/head)
            def emit_v(t):
                vht = v_sb[t][:, :].rearrange("p (h s) -> p h s", s=HD + 1)
                for c0, w in ((0, 512), (512, 256)):
                    pss = psG.tile([128, 512], f32, tag="g", name="v_ps")
                    for k in range(KC):
                        nc.tensor.matmul(
                            pss[:, 0:w], xTap(k, t * 128, (t + 1) * 128),
                            W16[k][:, 2 * C + c0:2 * C + c0 + w],
                            start=(k == 0), stop=(k == KC - 1))
                    nh = w // HD
                    h0 = c0 // HD
                    # strided 3-D out is full-rate on the scalar engine but
                    # pathological on DVE; bias not needed here (see header)
                    nc.scalar.copy(
                        vht[:, h0:h0 + nh, 0:HD],
                        pss[:, 0:w].rearrange("p (h s) -> p h s", s=HD))

            # ---- attention ----
            yT = [big.tile([128, T], f16, tag=f"yT_{m}", name=f"yT_{m}")
                  for m in range(KC)]

            def emit_pair(g, pr, den_on_scalar=False):
                """S -> exp/mask -> AV for head pair (2pr, 2pr+1), query
                group g. Software-pipelined: AV for chunk j issues after S
                for chunk j+1. Returns norm closure to schedule later."""
                hA, hB = 2 * pr, 2 * pr + 1
                nch = 4 * g + 4
                psyA = psY.tile([65, GW], f32, tag="y", name="psyA")
                psyB = psY.tile([65, GW], f32, tag="y", name="psyB")
                Es = []

                def emit_S(j):
                    cd = j - 4 * g
                    c0 = max(cd, 0) * 128
                    psS = psA.tile([128, 2, GW], f32, tag="s", name="psS")
                    for half, p0 in ((0, 0), (1, 64)):
                        nc.tensor.matmul(
                            psS[:, half, c0:GW],
                            qkT[6 + pr][p0:p0 + 64, j * 128:(j + 1) * 128],
                            qkT[pr][p0:p0 + 64, g * GW + c0:(g + 1) * GW],
                            start=True, stop=True)
                    E2 = ep.tile([128, 2, GW], f16, tag="e", name="E2")
                    nc.scalar.activation(
                        E2[:, :, c0:GW], psS[:, :, c0:GW], Exp, scale=SCALE)
                    if cd >= 0:
                        nc.gpsimd.affine_select(
                            out=E2[:, :, c0:c0 + 128], in_=E2[:, :, c0:c0 + 128],
                            compare_op=mybir.AluOpType.is_ge, fill=0.0,
                            base=0, pattern=[[0, 2], [1, 128]],
                            channel_multiplier=-1)
                    Es.append((E2, c0))

                def emit_AV(j):
                    E2, c0 = Es[j]
                    nc.tensor.matmul(
                        psyA[:, c0:GW],
                        v_sb[j][:, hA * (HD + 1):(hA + 1) * (HD + 1)],
                        E2[:, 0, c0:GW], start=(j == 0), stop=(j == nch - 1),
                        skip_group_check=True)
                    nc.tensor.matmul(
                        psyB[:, c0:GW],
                        v_sb[j][:, hB * (HD + 1):(hB + 1) * (HD + 1)],
                        E2[:, 1, c0:GW], start=(j == 0), stop=(j == nch - 1),
                        skip_group_check=True)

                for j in range(nch):
                    emit_S(j)
                    if j >= 1:
                        emit_AV(j - 1)
                emit_AV(nch - 1)

                recs = []
                for h, psy in ((hA, psyA), (hB, psyB)):
                    rec32 = recp.tile([1, 2, GW], f32, tag="rec32",
                                      name="rec32")
                    # custom-DVE recip can't read PSUM and only works at
                    # base partition 0: stage the denom row first
                    if den_on_scalar:
                        nc.scalar.copy(rec32[0:1, 0, :], psy[64:65, :])
                    else:
                        nc.vector.tensor_copy(rec32[0:1, 0, :], psy[64:65, :])
                    nc.vector.reciprocal_approx_fast(
                        rec32[0:1, 1, :], rec32[0:1, 0, :])
                    recs.append((h, psy, rec32))

                def norm():
                    for h, psy, rec32 in recs:
                        psb32 = recp.tile([64, GW], f32, tag="psb32",
                                          name="psb32")
                        nc.gpsimd.partition_broadcast(
                            psb32[:, :], rec32[0:1, 1, :], channels=64)
                        qt, qp = h // 2, (h % 2) * 64
                        nc.vector.tensor_mul(
                            yT[qt][qp:qp + 64, g * GW:(g + 1) * GW],
                            psy[0:64, :], psb32[:, :])
                return norm

            # ---- W_proj conversion + output bias tile, interleaved into
            # phase B below (so the vector queue isn't blocked behind the
            # late W_proj DMA but the work still precedes phase C)
            Wp16 = [wp.tile([128, C], f16, tag=f"Wp16_{k}", name=f"Wp16_{k}")
                    for k in range(KC)]
            bv16col = consts.tile([128, KC], f16, tag="bv16col")
            Bout = consts.tile([128, C], f16, tag="Bout")
            row16 = consts.tile([1, C], f16, tag="row16")

            def emit_wp(ks):
                for k in ks:
                    nc.vector.tensor_copy(Wp16[k][:, :], wp32s[k][:, :])

            def emit_bout_rps():
                for c0, w in ((0, 512), (512, 256)):
                    psr = psG.tile([1, 512], f32, tag="g", name="rps")
                    for k in range(KC):
                        nc.tensor.matmul(
                            psr[:, 0:w], bv16col[:, k:k + 1],
                            Wp16[k][:, c0:c0 + w],
                            start=(k == 0), stop=(k == KC - 1))
                    nc.vector.tensor_add(
                        row16[:, c0:c0 + w], psr[:, 0:w], bp32[:, c0:c0 + w])

            def emit_bout_bcast():
                for c0, w in ((0, 512), (512, 256)):
                    psb = psG.tile([128, 512], f32, tag="g", name="bbc")
                    nc.tensor.matmul(
                        psb[:, 0:w], ones16[0:1, :], row16[0:1, c0:c0 + w],
                        start=True, stop=True)
                    nc.vector.tensor_copy(Bout[:, c0:c0 + w], psb[:, 0:w])

            # ---- schedule: phase B = qkv GEMM + attention group 0 ----
            for t in range(4):
                emit_v(t)
            for pr in range(6):
                emit_qk(pr, 0)
                emit_qk(6 + pr, 0)
                norm = emit_pair(0, pr)
                emit_qk(pr, 1, on_vector=True)
                if pr < 4:
                    emit_v(4 + pr)
                elif pr == 5:
                    emit_bout_rps()
                norm()
                emit_qk(6 + pr, 1, on_vector=True)
                if pr == 3:
                    emit_wp(range(0, 3))
                    nc.vector.tensor_copy(bv16col[:, :], bcol[:, 12:18])
                elif pr == 4:
                    emit_wp(range(3, 6))
                elif pr == 5:
                    emit_bout_bcast()

            # ---- output projection ----
            proj_osb = {}

            def emit_proj(t, chunks=((0, 512), (512, 256))):
                if t not in proj_osb:
                    proj_osb[t] = outp.tile([128, C], f32, tag="o",
                                            name="o_sb")
                o_sb = proj_osb[t]
                for c0, w in chunks:
                    psO = psG.tile([128, 512], f32, tag="g", name="o_ps")
                    for k in range(KC):
                        nc.tensor.matmul(
                            psO[:, 0:w], yT[k][:, t * 128:(t + 1) * 128],
                            Wp16[k][:, c0:c0 + w],
                            start=(k == 0), stop=(k == KC - 1))
                    nc.vector.tensor_add(
                        o_sb[:, c0:c0 + w], psO[:, 0:w], Bout[:, c0:c0 + w])
                    # per-chunk output DMA: the large chunk's transfer
                    # overlaps the small chunk's matmuls
                    nc.sync.dma_start(
                        out=out[t * 128:(t + 1) * 128, c0:c0 + w],
                        in_=o_sb[:, c0:c0 + w])

            # ---- phase C: attention group 1 with projections as filler,
            # proj3 split across pr3/pr4 and proj4's first 5 k-chunks
            # prefilled at pr5 so every norm chain has PE cover
            pre45 = {}

            def proj_cont(t, ks, tiles=None):
                if tiles is not None:
                    proj_osb[t] = outp.tile([128, C], f32, tag="o",
                                            name="o_sb")
                for c0, w in ((0, 512), (512, 256)):
                    psO = tiles[c0] if tiles is not None else pre45[(t, c0)]
                    for k in ks:
                        nc.tensor.matmul(
                            psO[0:128, 0:w], yT[k][:, t * 128:(t + 1) * 128],
                            Wp16[k][:, c0:c0 + w],
                            start=(k == 0), stop=False)
                    pre45[(t, c0)] = psO

            def proj_finish(t, kstart):
                o_sb = proj_osb[t]
                for c0, w in ((0, 512), (512, 256)):
                    psO = pre45[(t, c0)]
                    for k in range(kstart, KC):
                        nc.tensor.matmul(
                            psO[0:128, 0:w], yT[k][:, t * 128:(t + 1) * 128],
                            Wp16[k][:, c0:c0 + w],
                            start=False, stop=(k == KC - 1))
                    nc.vector.tensor_add(
                        o_sb[:, c0:c0 + w], psO[0:128, 0:w], Bout[:, c0:c0 + w])
                    nc.sync.dma_start(
                        out=out[t * 128:(t + 1) * 128, c0:c0 + w],
                        in_=o_sb[:, c0:c0 + w])

            for pr in range(6):
                norm = emit_pair(1, pr, den_on_scalar=(pr >= 4))
                if pr < 3:
                    emit_proj(pr)
                elif pr == 3:
                    emit_proj(3)
                elif pr == 4:
                    tiles4 = {c0: psG.tile([128, 512], f32, tag="g",
                                           name="o_ps")[:, :]
                              for c0 in (0, 512)}
                    proj_cont(4, range(0, 4), tiles4)
                elif pr == 5:
                    # cover the last norm chain with independent proj work
                    proj_cont(4, range(4, 5))
                    psA5 = psA.tile([128, 2, GW], f32, tag="s", name="psS")
                    proj_cont(5, range(0, 5),
                              {0: psA5[:, 0, :], 512: psA5[:, 1, :]})
                    psA6 = psA.tile([128, 2, GW], f32, tag="s", name="psS")
                    proj_cont(6, range(0, 5),
                              {0: psA6[:, 0, :], 512: psA6[:, 1, :]})
                norm()
            proj_finish(4, 5)
            proj_finish(5, 5)
            proj_finish(6, 5)
            emit_proj(7)

    nc.finalize()
    return nc


_CACHE = {}


def _get_nc():
    if "nc" not in _CACHE:
        _CACHE["nc"] = build_nc()
    return _CACHE["nc"]


def run(inputs, trace=False):
    nc = _get_nc()
    x = np.asarray(inputs["x"], dtype=np.float32)
    in_maps = [
        {
            "x": np.ascontiguousarray(x[i]),
            "W_attn": np.asarray(inputs["W_attn"], dtype=np.float32),
            "b_attn": np.asarray(inputs["b_attn"], dtype=np.float32),
            "W_proj": np.asarray(inputs["W_proj"], dtype=np.float32),
            "b_proj": np.asarray(inputs["b_proj"], dtype=np.float32),
        }
        for i in range(B)
    ]
    res = run_bass_kernel_spmd(nc, in_maps, core_ids=list(range(B)), trace=trace)
    y = np.stack([res.results[i]["out"] for i in range(B)], axis=0)
    return y, res


def kernel(**inputs):
    y, _ = run(inputs, trace=False)
    return y


# revision 57
# speedup vs baseline: 1.0103x; 1.0103x over previous
"""Causal self-attention (transformer block) on 8 trn2 NeuronCores.

Data-parallel over batch: core i processes batch element i (B=8).
Per-core dataflow (T=1024, C=768, H=12 heads, hd=64), fp16 matmul
operands with fp32 PSUM accumulation:

  x [T,C] --PE transpose (f32)--> xT [C,T] f16    (feature-major)
  qkT[m]  = W_attn[:,m].T @ xT + b  [1536,T]      (feature-major q,k)
  v[t]    = xT[t].T @ W_attn[:,v]   [T,768+ones]  (row-major; v-bias is
            folded into the output bias: softmax rows sum to 1, so
            y = sum a*(v+b_v) = sum a*v + b_v, and b_v^T W_proj is a
            constant row added to the projection bias)
  S^T[j,i] = sum_d kT[d,j] qT[d,i]                (scores transposed,
             only causally-live 128-col strips computed)
  E = exp(S^T*scale); diag blocks masked in-place (gpsimd affine_select)
  psY += [v_j | 1].T @ E_j   (partial width; row 64 = softmax denom)
  yT = psY[0:64] * bcast(1/psY[64])               (per-half-pair norm)
  out[t] = yT[:,t].T @ W_proj + Bout, DMA out

DMA order W_attn -> x -> W_proj is enforced by a shared staging ring
(slot reuse serializes), so the qk GEMM starts ~10us earlier than with
bandwidth-shared concurrent loads.
"""
import numpy as np

import concourse.bass as bass
import concourse.tile as tile
from concourse import bacc, mybir
from concourse.bass_utils import run_bass_kernel_spmd
from concourse.masks import make_identity

f32 = mybir.dt.float32
f16 = mybir.dt.float16
Exp = mybir.ActivationFunctionType.Exp
Ident = mybir.ActivationFunctionType.Identity

B = 8
T = 1024
C = 768
H = 12
HD = 64
SCALE = HD ** -0.5
KC = C // 128        # 6 feature chunks
MT = T // 128        # 8 token tiles
GW = 512             # Tq group width
NG = T // GW         # 2 groups
VW = H * (HD + 1)    # v tile width incl. ones column (780)


def build_nc():
    nc = bacc.Bacc(None)
    x = nc.dram_tensor("x", [T, C], f32, kind="ExternalInput")
    W_attn = nc.dram_tensor("W_attn", [C, 3 * C], f32, kind="ExternalInput")
    b_attn = nc.dram_tensor("b_attn", [3 * C], f32, kind="ExternalInput")
    W_proj = nc.dram_tensor("W_proj", [C, C], f32, kind="ExternalInput")
    b_proj = nc.dram_tensor("b_proj", [C], f32, kind="ExternalInput")
    out = nc.dram_tensor("out", [T, C], f32, kind="ExternalOutput")

    with tile.TileContext(nc) as tc:
        with (
            tc.tile_pool(name="consts", bufs=1) as consts,
            tc.tile_pool(name="stage", bufs=1) as stage,
            tc.tile_pool(name="stageW", bufs=4) as stageW,
            tc.tile_pool(name="stageX", bufs=3) as stageX,
            tc.tile_pool(name="wq", bufs=1) as wq,
            tc.tile_pool(name="wp", bufs=1) as wp,
            tc.tile_pool(name="big", bufs=1) as big,
            tc.tile_pool(name="ep", bufs=6) as ep,
            tc.tile_pool(name="recp", bufs=2) as recp,
            tc.tile_pool(name="outp", bufs=2) as outp,
            tc.tile_pool(name="psG", bufs=2, space="PSUM") as psG,
            tc.tile_pool(name="psA", bufs=2, space="PSUM") as psA,
            tc.tile_pool(name="psY", bufs=2, space="PSUM") as psY,
        ):
            # ---- constants ----
            ident32 = consts.tile([128, 128], f32, tag="ident32")
            make_identity(nc, ident32[:, :])
            ones16 = consts.tile([1, 128], f16, tag="ones16")
            nc.vector.memset(ones16[:, :], 1.0)
            ones65 = consts.tile([65, 64], f16, tag="ones65")
            nc.vector.memset(ones65[:, :], 1.0)

            # qk bias, one column per 128-feature chunk (per-partition add).
            # Loaded contiguously as [18,128] + one PE transpose — the
            # direct [128,18] load would be 2304 scattered 4-byte DMA
            # descriptors ahead of the weight DMAs.
            NB = 3 * C // 128
            bst = stage.tile([NB, 128], f32, tag="bst")
            nc.sync.dma_start(
                out=bst[:, :],
                in_=b_attn.ap().rearrange("(m p) -> m p", p=128))
            bcol = consts.tile([128, NB], f32, tag="bcol")
            psbt = psG.tile([128, NB], f32, tag="g", name="psbt")
            nc.tensor.transpose(psbt[:, :], bst[:, :], ident32[0:NB, 0:NB])
            nc.scalar.copy(bcol[:, :], psbt[:, :])
            bp32 = stage.tile([1, C], f32, tag="bp32")
            nc.sync.dma_start(out=bp32[:, :], in_=b_proj.ap().rearrange("(a d) -> a d", a=1))

            # ---- v tiles + their ones columns first: no data deps, so the
            # gpsimd queue isn't blocked behind DMA-gated weight converts
            v_sb = [big.tile([128, VW], f16, tag=f"v_{t}", name=f"v_{t}")
                    for t in range(MT)]
            for t in range(MT):
                vht_ = v_sb[t][:, :].rearrange("p (h s) -> p h s", s=HD + 1)
                nc.gpsimd.memset(vht_[:, :, HD:HD + 1], 1.0)

            # ---- staged loads (W first in issue order, then x, then W_proj)
            W16 = []
            # xT split in half-T tiles so group-0 consumers don't wait on
            # token tiles 4..7
            xTh = [big.tile([128, KC, T // 2], f16, tag=f"xT{i}", name=f"xT{i}")
                   for i in range(2)]

            def xTap(k, lo, hi):
                if hi <= T // 2:
                    return xTh[0][:, k, lo:hi]
                return xTh[1][:, k, lo - T // 2:hi - T // 2]

            for k in range(KC):
                w32 = stageW.tile([128, 3 * C], f32, tag="w32", name=f"w32_{k}")
                nc.sync.dma_start(out=w32[:, :], in_=W_attn[k * 128:(k + 1) * 128, :])
                wt = wq.tile([128, 3 * C], f16, tag=f"W16_{k}", name=f"W16_{k}")
                nc.scalar.copy(wt[:, 0:3 * C // 2], w32[:, 0:3 * C // 2])
                nc.vector.tensor_copy(wt[:, 3 * C // 2:], w32[:, 3 * C // 2:])
                W16.append(wt)
            def emit_transpose(t, x32):
                for half in range(2):
                    pst = psG.tile([128, 3, 128], f32, tag="g", name="pst")
                    for c in range(3):
                        cc = half * 3 + c
                        nc.tensor.transpose(
                            pst[:, c, :], x32[:, cc * 128:(cc + 1) * 128],
                            ident32[:, :])
                    tt = (t % 4) * 128
                    dst = xTh[t // 4][:, half * 3:half * 3 + 3, tt:tt + 128]
                    if half == 0 and t < 4:
                        nc.scalar.copy(dst, pst[:, :, :])
                    else:
                        nc.vector.tensor_copy(dst, pst[:, :, :])

            x32_late = {}
            for t in range(MT):
                # tiles 4..7 go through the W ring: the slot-reuse dep holds
                # their DMAs back so W_attn + x0..3 own the early bandwidth.
                # Their transposes are deferred into phase B so the in-order
                # PE queue isn't blocked on the late DMAs.
                pool = stageX if t < 4 else stageW
                x32 = pool.tile([128, C], f32,
                                tag="x32" if t < 4 else "w32",
                                name=f"x32_{t}")
                nc.sync.dma_start(out=x32[:, :], in_=x[t * 128:(t + 1) * 128, :])
                emit_transpose(t, x32)
            # W_proj staged through the same ring as W_attn: the slot-reuse
            # dependency keeps these DMAs off the wire until the W_attn
            # conversions are done (W_proj isn't needed until phase C).
            wp32s = []
            for k in range(KC):
                wp32 = stageW.tile([128, C], f32, tag="w32", name=f"wp32_{k}")
                nc.sync.dma_start(out=wp32[:, :], in_=W_proj[k * 128:(k + 1) * 128, :])
                wp32s.append(wp32)

            # ---- qk^T GEMM: qkT[m] [128, T] f16, m 0..5 = q, 6..11 = k
            qkT = [big.tile([128, T], f16, tag=f"qkT_{m}", name=f"qkT_{m}")
                   for m in range(12)]

            def emit_qk(m, n, on_vector=False):
                ps = psG.tile([128, 512], f32, tag="g", name="qk_ps")
                for k in range(KC):
                    nc.tensor.matmul(
                        ps[:, :], W16[k][:, m * 128:(m + 1) * 128],
                        xTap(k, n * 512, (n + 1) * 512),
                        start=(k == 0), stop=(k == KC - 1))
                dst = qkT[m][:, n * 512:(n + 1) * 512]
                if on_vector:
                    nc.vector.tensor_scalar_add(dst, ps[:, :], bcol[:, m:m + 1])
                else:
                    nc.scalar.activation(dst, ps[:, :], Ident, bias=bcol[:, m:m + 1])

            # ---- v rows: v_sb[t] [128, 780] f16 (64 cols + ones col/head)
            def emit_v(t):
                vht = v_sb[t][:, :].rearrange("p (h s) -> p h s", s=HD + 1)
                for c0, w in ((0, 512), (512, 256)):
                    pss = psG.tile([128, 512], f32, tag="g", name="v_ps")
                    for k in range(KC):
                        nc.tensor.matmul(
                            pss[:, 0:w], xTap(k, t * 128, (t + 1) * 128),
                            W16[k][:, 2 * C + c0:2 * C + c0 + w],
                            start=(k == 0), stop=(k == KC - 1))
                    nh = w // HD
                    h0 = c0 // HD
                    # strided 3-D out is full-rate on the scalar engine but
                    # pathological on DVE; bias not needed here (see header)
                    nc.scalar.copy(
                        vht[:, h0:h0 + nh, 0:HD],
                        pss[:, 0:w].rearrange("p (h s) -> p h s", s=HD))

            # ---- attention ----
            yT = [big.tile([128, T], f16, tag=f"yT_{m}", name=f"yT_{m}")
                  for m in range(KC)]

            def emit_pair(g, pr, den_on_scalar=False):
                """S -> exp/mask -> AV for head pair (2pr, 2pr+1), query
                group g. Software-pipelined: AV for chunk j issues after S
                for chunk j+1. Returns norm closure to schedule later."""
                hA, hB = 2 * pr, 2 * pr + 1
                nch = 4 * g + 4
                psyA = psY.tile([65, GW], f32, tag="y", name="psyA")
                psyB = psY.tile([65, GW], f32, tag="y", name="psyB")
                Es = []

                def emit_S(j):
                    cd = j - 4 * g
                    c0 = max(cd, 0) * 128
                    psS = psA.tile([128, 2, GW], f32, tag="s", name="psS")
                    for half, p0 in ((0, 0), (1, 64)):
                        nc.tensor.matmul(
                            psS[:, half, c0:GW],
                            qkT[6 + pr][p0:p0 + 64, j * 128:(j + 1) * 128],
                            qkT[pr][p0:p0 + 64, g * GW + c0:(g + 1) * GW],
                            start=True, stop=True)
                    E2 = ep.tile([128, 2, GW], f16, tag="e", name="E2")
                    nc.scalar.activation(
                        E2[:, :, c0:GW], psS[:, :, c0:GW], Exp, scale=SCALE)
                    if cd >= 0:
                        nc.gpsimd.affine_select(
                            out=E2[:, :, c0:c0 + 128], in_=E2[:, :, c0:c0 + 128],
                            compare_op=mybir.AluOpType.is_ge, fill=0.0,
                            base=0, pattern=[[0, 2], [1, 128]],
                            channel_multiplier=-1)
                    Es.append((E2, c0))

                def emit_AV(j):
                    E2, c0 = Es[j]
                    nc.tensor.matmul(
                        psyA[:, c0:GW],
                        v_sb[j][:, hA * (HD + 1):(hA + 1) * (HD + 1)],
                        E2[:, 0, c0:GW], start=(j == 0), stop=(j == nch - 1),
                        skip_group_check=True)
                    nc.tensor.matmul(
                        psyB[:, c0:GW],
                        v_sb[j][:, hB * (HD + 1):(hB + 1) * (HD + 1)],
                        E2[:, 1, c0:GW], start=(j == 0), stop=(j == nch - 1),
                        skip_group_check=True)

                for j in range(nch):
                    emit_S(j)
                    if j >= 1:
                        emit_AV(j - 1)
                emit_AV(nch - 1)

                recs = []
                for h, psy in ((hA, psyA), (hB, psyB)):
                    rec32 = recp.tile([1, 2, GW], f32, tag="rec32",
                                      name="rec32")
                    # custom-DVE recip can't read PSUM and only works at
                    # base partition 0: stage the denom row first
                    if den_on_scalar:
                        nc.scalar.copy(rec32[0:1, 0, :], psy[64:65, :])
                    else:
                        nc.vector.tensor_copy(rec32[0:1, 0, :], psy[64:65, :])
                    nc.vector.reciprocal_approx_fast(
                        rec32[0:1, 1, :], rec32[0:1, 0, :])
                    recs.append((h, psy, rec32))

                def norm():
                    for h, psy, rec32 in recs:
                        psb32 = recp.tile([64, GW], f32, tag="psb32",
                                          name="psb32")
                        nc.gpsimd.partition_broadcast(
                            psb32[:, :], rec32[0:1, 1, :], channels=64)
                        qt, qp = h // 2, (h % 2) * 64
                        nc.vector.tensor_mul(
                            yT[qt][qp:qp + 64, g * GW:(g + 1) * GW],
                            psy[0:64, :], psb32[:, :])
                return norm

            # ---- W_proj conversion + output bias tile, interleaved into
            # phase B below (so the vector queue isn't blocked behind the
            # late W_proj DMA but the work still precedes phase C)
            Wp16 = [wp.tile([128, C], f16, tag=f"Wp16_{k}", name=f"Wp16_{k}")
                    for k in range(KC)]
            bv16col = consts.tile([128, KC], f16, tag="bv16col")
            Bout = consts.tile([128, C], f16, tag="Bout")
            row16 = consts.tile([1, C], f16, tag="row16")

            def emit_wp(ks):
                for k in ks:
                    nc.vector.tensor_copy(Wp16[k][:, :], wp32s[k][:, :])

            def emit_bout_rps():
                for c0, w in ((0, 512), (512, 256)):
                    psr = psG.tile([1, 512], f32, tag="g", name="rps")
                    for k in range(KC):
                        nc.tensor.matmul(
                            psr[:, 0:w], bv16col[:, k:k + 1],
                            Wp16[k][:, c0:c0 + w],
                            start=(k == 0), stop=(k == KC - 1))
                    nc.vector.tensor_add(
                        row16[:, c0:c0 + w], psr[:, 0:w], bp32[:, c0:c0 + w])

            def emit_bout_bcast():
                for c0, w in ((0, 512), (512, 256)):
                    psb = psG.tile([128, 512], f32, tag="g", name="bbc")
                    nc.tensor.matmul(
                        psb[:, 0:w], ones16[0:1, :], row16[0:1, c0:c0 + w],
                        start=True, stop=True)
                    nc.vector.tensor_copy(Bout[:, c0:c0 + w], psb[:, 0:w])

            # ---- schedule: phase B = qkv GEMM + attention group 0 ----
            for t in range(4):
                emit_v(t)
            for pr in range(6):
                emit_qk(pr, 0)
                emit_qk(6 + pr, 0)
                norm = emit_pair(0, pr)
                emit_qk(pr, 1, on_vector=True)
                if pr < 4:
                    emit_v(4 + pr)
                elif pr == 5:
                    emit_bout_rps()
                norm()
                emit_qk(6 + pr, 1, on_vector=True)
                if pr == 1:
                    nc.vector.tensor_copy(bv16col[:, :], bcol[:, 12:18])
                if pr in (1, 2, 3):
                    emit_wp(range((pr - 1) * 2, pr * 2))
                elif pr == 5:
                    emit_bout_bcast()

            # ---- output projection ----
            proj_osb = {}

            def emit_proj(t, chunks=((0, 512), (512, 256))):
                if t not in proj_osb:
                    proj_osb[t] = outp.tile([128, C], f32, tag="o",
                                            name="o_sb")
                o_sb = proj_osb[t]
                for c0, w in chunks:
                    psO = psG.tile([128, 512], f32, tag="g", name="o_ps")
                    for k in range(KC):
                        nc.tensor.matmul(
                            psO[:, 0:w], yT[k][:, t * 128:(t + 1) * 128],
                            Wp16[k][:, c0:c0 + w],
                            start=(k == 0), stop=(k == KC - 1))
                    nc.vector.tensor_add(
                        o_sb[:, c0:c0 + w], psO[:, 0:w], Bout[:, c0:c0 + w])
                    # per-chunk output DMA: the large chunk's transfer
                    # overlaps the small chunk's matmuls
                    nc.sync.dma_start(
                        out=out[t * 128:(t + 1) * 128, c0:c0 + w],
                        in_=o_sb[:, c0:c0 + w])

            # ---- phase C: attention group 1 with projections as filler,
            # proj3 split across pr3/pr4 and proj4's first 5 k-chunks
            # prefilled at pr5 so every norm chain has PE cover
            pre45 = {}

            def proj_cont(t, ks, tiles=None):
                if tiles is not None:
                    proj_osb[t] = outp.tile([128, C], f32, tag="o",
                                            name="o_sb")
                for c0, w in ((0, 512), (512, 256)):
                    psO = tiles[c0] if tiles is not None else pre45[(t, c0)]
                    for k in ks:
                        nc.tensor.matmul(
                            psO[0:128, 0:w], yT[k][:, t * 128:(t + 1) * 128],
                            Wp16[k][:, c0:c0 + w],
                            start=(k == 0), stop=False)
                    pre45[(t, c0)] = psO

            def proj_finish(t, kstart):
                o_sb = proj_osb[t]
                for c0, w in ((0, 512), (512, 256)):
                    psO = pre45[(t, c0)]
                    for k in range(kstart, KC):
                        nc.tensor.matmul(
                            psO[0:128, 0:w], yT[k][:, t * 128:(t + 1) * 128],
                            Wp16[k][:, c0:c0 + w],
                            start=False, stop=(k == KC - 1))
                    nc.vector.tensor_add(
                        o_sb[:, c0:c0 + w], psO[0:128, 0:w], Bout[:, c0:c0 + w])
                    nc.sync.dma_start(
                        out=out[t * 128:(t + 1) * 128, c0:c0 + w],
                        in_=o_sb[:, c0:c0 + w])

            for pr in range(6):
                norm = emit_pair(1, pr, den_on_scalar=(pr >= 4))
                if pr < 3:
                    emit_proj(pr)
                elif pr == 3:
                    emit_proj(3)
                elif pr == 4:
                    tiles4 = {c0: psG.tile([128, 512], f32, tag="g",
                                           name="o_ps")[:, :]
                              for c0 in (0, 512)}
                    proj_cont(4, range(0, 4), tiles4)
                elif pr == 5:
                    # cover the last norm chain with independent proj work
                    proj_cont(4, range(4, 5))
                    psA5 = psA.tile([128, 2, GW], f32, tag="s", name="psS")
                    proj_cont(5, range(0, 5),
                              {0: psA5[:, 0, :], 512: psA5[:, 1, :]})
                    psA6 = psA.tile([128, 2, GW], f32, tag="s", name="psS")
                    proj_cont(6, range(0, 5),
                              {0: psA6[:, 0, :], 512: psA6[:, 1, :]})
                norm()
            proj_finish(4, 5)
            proj_finish(5, 5)
            proj_finish(6, 5)
            emit_proj(7)

    nc.finalize()
    return nc


_CACHE = {}


def _get_nc():
    if "nc" not in _CACHE:
        _CACHE["nc"] = build_nc()
    return _CACHE["nc"]


def run(inputs, trace=False):
    nc = _get_nc()
    x = np.asarray(inputs["x"], dtype=np.float32)
    in_maps = [
        {
            "x": np.ascontiguousarray(x[i]),
            "W_attn": np.asarray(inputs["W_attn"], dtype=np.float32),
            "b_attn": np.asarray(inputs["b_attn"], dtype=np.float32),
            "W_proj": np.asarray(inputs["W_proj"], dtype=np.float32),
            "b_proj": np.asarray(inputs["b_proj"], dtype=np.float32),
        }
        for i in range(B)
    ]
    res = run_bass_kernel_spmd(nc, in_maps, core_ids=list(range(B)), trace=trace)
    y = np.stack([res.results[i]["out"] for i in range(B)], axis=0)
    return y, res


def kernel(**inputs):
    y, _ = run(inputs, trace=False)
    return y
